# revision 16
# baseline (speedup 1.0000x reference)
"""Distributed Trainium2 kernel: LayerNorm -> QKV -> causal MHA -> out-proj.

Sharding (8 cores):
  - LayerNorm + final projection: token-parallel, strip-interleaved: core c
    owns token strips {1024*r + 128*c + [0,128) : r=0..3} (4096 tokens ->
    512/core).  Strip interleaving lets the post-LN AllGather be split into
    4 pipelined 2MiB collectives, each gathering one contiguous 1024-token
    block, so QKV/attention compute starts after the first lands.
  - Attention + QKV: head-parallel (16 heads -> 2/core).
  - Comms: 4x AllGather of xn^T strips (bf16) after LN; 2x AllToAll of
    per-head attention output (one per 2048-token half) re-shards the
    inner-dim contraction to token-parallel; no AllReduce needed.

Layout notes:
  - All activations are kept TRANSPOSED ([feature, token]) so every matmul
    contraction runs over the partition axis.  S is computed transposed
    (S^T[j,i] = k_j . q_i); softmax sums come from an appended ones-column
    on V (m=65 matmul); the dh^-0.5 scale rides the exp activation's free
    affine.  Streams are causally trimmed at 128 granularity; the single
    partial diagonal 128x128 tile is zeroed with a triangle mask post-exp.
  - The two heads' K=64 S-matmuls are emitted back-to-back at row-disjoint
    tile_position (0,0)/(64,0) so they co-execute on the PE array.
  - gamma/beta are folded host-side into w_qkv (row scale) and per-feature
    biases (beta @ w_qkv), so LN on-device is just (x-mu)*rstd.
  - Matmul inputs are bf16 (4x the fp32 TensorE rate); accumulation fp32.
  - Attention groups are interleaved into the QKV token-chunk loop so
    ScalarE exp work overlaps TensorE QKV matmuls and the PE stream stays
    dense (HAM-warm).  The first AllToAll + its out-projection are
    interleaved into the attention tail.
"""

import numpy as np
import ml_dtypes

import concourse.bass as bass
import concourse.tile as tile
from concourse import bacc, mybir
from concourse.bass import ds, ts
from concourse.bass_utils import run_bass_kernel_spmd
from concourse.masks import make_identity

B, N, D = 2, 2048, 1024
HEADS, DH = 16, 64
INNER = HEADS * DH          # 1024
NCORES = 8
T = B * N                   # 4096 tokens
TS = T // NCORES            # 512 tokens per core
HPC = HEADS // NCORES       # 2 heads per core
SCALE = float(DH) ** -0.5   # 0.125
EPS = 1e-5

FP = mybir.dt.float32
BF = mybir.dt.bfloat16

KT = D // 128               # 8 contraction tiles of 128 over dim
TCH = T // 512              # 8 token chunks of 512
ICB = N // 512              # 4 i-chunks of 512 per batch
JTB = N // 128              # 16 j-tiles of 128 per batch
NSTRIP = 4                  # AllGather rounds; round r = tokens [1024r,1024r+1024)


def build():
    nc = bacc.Bacc("TRN2", target_bir_lowering=False, debug=False,
                   num_devices=NCORES)

    x_sh = nc.dram_tensor("x_shard", [TS, D], FP, kind="ExternalInput")
    wq_t = nc.dram_tensor("wq", [D, HPC * DH], BF, kind="ExternalInput")
    wk_t = nc.dram_tensor("wk", [D, HPC * DH], BF, kind="ExternalInput")
    wv_t = nc.dram_tensor("wv", [D, HPC * DH], BF, kind="ExternalInput")
    bq_t = nc.dram_tensor("bq", [HPC * DH, 1], FP, kind="ExternalInput")
    bk_t = nc.dram_tensor("bk", [HPC * DH, 1], FP, kind="ExternalInput")
    bv_t = nc.dram_tensor("bv", [HPC * DH, 1], FP, kind="ExternalInput")
    wo_t = nc.dram_tensor("w_out", [INNER, D], BF, kind="ExternalInput")
    out_sh = nc.dram_tensor("out_shard", [TS, D], FP, kind="ExternalOutput")

    with tile.TileContext(nc) as tc:
        _body(nc, tc, x_sh, wq_t, wk_t, wv_t, bq_t, bk_t, bv_t, wo_t, out_sh)

    nc.compile()
    return nc


def _att_thunks(nc, b, ic, kTt, qT, vhat, tri, outT,
                s_ps, av_ps, espool, smallp):
    """Attention for query chunk (b, ic) as a list of PE-ordered thunks.

    Each jp step emits the two heads' S matmuls in a single thunk pair so
    the row-disjoint (tile_position 0 / 64) K=64 matmuls stay adjacent in
    the PE queue and co-execute.  S-matmuls for step jp are emitted before
    the AV-matmuls of step jp-1 so the PE never waits on ACT exp latency.
    """
    q_idx = b * ICB + ic
    njt = 4 * (ic + 1)
    av = [av_ps.tile([128, 512], FP, tag=f"av{h}", name=f"av{h}_{q_idx}")
          for h in range(HPC)]
    es = {}

    def i0_of(jt):
        m = jt - 4 * ic
        return 128 * m if m > 0 else 0

    def s_pair(jp, u):
        def run():
            if u == 0:
                for h in range(HPC):
                    sx = s_ps.tile([128, 1024], FP, tag="sx",
                                   name=f"sx{h}_{q_idx}_{jp}")
                    es[(h, jp)] = (sx, None)
            jt = 2 * jp + u
            tq = b * ICB + jt // 4
            jo = 128 * (jt % 4)
            i0 = i0_of(jt)
            for h in range(HPC):
                sx, _ = es[(h, jp)]
                nc.tensor.matmul(
                    sx[:, ds(512 * u + i0, 512 - i0)],
                    kTt[ds(64 * h, 64), tq, ds(jo, 128)],
                    qT[ds(64 * h, 64), q_idx, ds(i0, 512 - i0)],
                    start=True, stop=True,
                    tile_position=(64 * h, 0))
        return run

    def exp_t(h, jp):
        def run():
            sx, _ = es[(h, jp)]
            e = espool.tile([128, 1024], BF, tag="es",
                            name=f"es{h}_{q_idx}_{jp}")
            es[(h, jp)] = (sx, e)
            i0s = [i0_of(2 * jp), i0_of(2 * jp + 1)]
            if i0s[0] == 0 and i0s[1] == 0:
                nc.scalar.activation(
                    out=e, in_=sx,
                    func=mybir.ActivationFunctionType.Exp, scale=SCALE)
            else:
                for u in range(2):
                    i0 = i0s[u]
                    nc.scalar.activation(
                        out=e[:, ds(512 * u + i0, 512 - i0)],
                        in_=sx[:, ds(512 * u + i0, 512 - i0)],
                        func=mybir.ActivationFunctionType.Exp, scale=SCALE)
        return run

    def av_mm(h, jp, u):
        def run():
            _, e = es[(h, jp)]
            jt = 2 * jp + u
            m = jt - 4 * ic
            i0 = i0_of(jt)
            if 0 <= m < 4:
                # zero the strict upper triangle of the diagonal 128x128 tile
                nc.vector.tensor_tensor(
                    out=e[:, ds(512 * u + 128 * m, 128)],
                    in0=e[:, ds(512 * u + 128 * m, 128)],
                    in1=tri,
                    op=mybir.AluOpType.mult)
            nc.tensor.matmul(
                av[h][0:65, ds(i0, 512 - i0)],
                vhat[:, b * JTB + jt, ds(65 * h, 65)],
                e[:, ds(512 * u + i0, 512 - i0)],
                start=(jt == 0), stop=(jt == njt - 1))
        return run

    def norm(h):
        def run():
            rsum = smallp.tile([1, 512], FP, tag="rsum", name=f"rs{h}_{q_idx}")
            nc.vector.tensor_copy(out=rsum, in_=av[h][64:65, :])
            rec = smallp.tile([1, 512], FP, tag="rec", name=f"rc{h}_{q_idx}")
            nc.vector.reciprocal_approx_fast(out=rec, in_=rsum)
            bc = smallp.tile([64, 512], FP, tag="bc", name=f"bc{h}_{q_idx}")
            nc.gpsimd.partition_broadcast(bc, rec)
            nc.vector.tensor_tensor(
                out=outT[h][:, ds(512 * q_idx, 512)],
                in0=av[h][0:64, :], in1=bc,
                op=mybir.AluOpType.mult)
        return run

    def s_group(jp):
        return [s_pair(jp, 0), s_pair(jp, 1), exp_t(0, jp), exp_t(1, jp)]

    def av_group(jp):
        return [av_mm(0, jp, 0), av_mm(0, jp, 1),
                av_mm(1, jp, 0), av_mm(1, jp, 1)]

    thunks = []
    nps = njt // 2
    thunks.extend(s_group(0))
    for jp in range(1, nps):
        thunks.extend(s_group(jp))
        thunks.extend(av_group(jp - 1))
    thunks.extend(av_group(nps - 1))
    thunks.append(norm(0))
    thunks.append(norm(1))
    return thunks


def _qkv_thunks(nc, tci, xt_u, wq_sb, wk_sb, wv_sb, bq_b, bk_b, bv_b,
                qT, kTt, vhat, qkv_ps, vst):
    """QKV projection for token chunk tci as a list of PE-ordered thunks."""
    thunks = []

    def mk_group(w_sb, nm):
        acc = qkv_ps.tile([128, 512], FP, tag="acc", name=f"acc{nm}_{tci}")

        def mm(k):
            def run():
                nc.tensor.matmul(acc, w_sb[:, k, :], xt_u[:, k, :],
                                 start=(k == 0), stop=(k == KT - 1))
            return run
        return acc, mm

    for w_sb, bias, dst, nm in ((wq_sb, bq_b, qT, "q"),
                                (wk_sb, bk_b, kTt, "k")):
        acc, mm = mk_group(w_sb, nm)
        for k in range(KT):
            thunks.append(mm(k))

        def copy(acc=acc, bias=bias, dst=dst, tci=tci):
            nc.vector.tensor_scalar(
                out=dst[:, tci, :], in0=acc, scalar1=bias, scalar2=None,
                op0=mybir.AluOpType.add)
        thunks.append(copy)

    accv, mmv = mk_group(wv_sb, "v")
    for k in range(KT):
        thunks.append(mmv(k))
    vs = vst.tile([128, 512], BF, tag="vs", name=f"vs_{tci}")

    def vcopy():
        nc.vector.tensor_scalar(
            out=vs, in0=accv, scalar1=bv_b, scalar2=None,
            op0=mybir.AluOpType.add)
    thunks.append(vcopy)

    def vtrans():
        def run():
            vstg = vst.tile([128, 4, 128], BF, tag="vstg", name=f"vstg_{tci}")
            nc.sync.dma_start_transpose(out=vstg, in_=vs)
            nc.vector.tensor_copy(out=vhat[:, ds(tci * 4, 4), 0:64],
                                  in_=vstg[:, :, 0:64])
            nc.vector.tensor_copy(out=vhat[:, ds(tci * 4, 4), 65:129],
                                  in_=vstg[:, :, 64:128])
        return run
    thunks.append(vtrans())
    return thunks


def _merge(primary, filler):
    """Interleave filler thunks evenly between primary thunks."""
    out = []
    np_, nf = len(primary), len(filler)
    fi = 0
    for i, p in enumerate(primary):
        out.append(p)
        want = (i + 1) * nf // np_
        while fi < want:
            out.append(filler[fi])
            fi += 1
    out.extend(filler[fi:])
    return out


def _body(nc, tc, x_sh, wq_t, wk_t, wv_t, bq_t, bk_t, bv_t, wo_t, out_sh):
    from contextlib import ExitStack
    ctx = ExitStack()
    with ctx:
        const = ctx.enter_context(tc.tile_pool(name="const", bufs=1))
        wpool = ctx.enter_context(tc.tile_pool(name="wpool", bufs=1))
        big = ctx.enter_context(tc.tile_pool(name="big", bufs=1))
        dram = ctx.enter_context(tc.tile_pool(name="dram", bufs=1, space="DRAM"))

        # ---------- constants ----------
        identity = const.tile([128, 128], BF)
        make_identity(nc, identity)

        # 0/1 lower-triangle mask for the one partial 128x128 diagonal tile
        # of S^T: keep [jj, ii] iff ii - jj >= 0.
        tri = const.tile([128, 128], BF)
        nc.gpsimd.memset(tri, 1.0)
        nc.gpsimd.affine_select(
            out=tri, in_=tri,
            compare_op=mybir.AluOpType.is_ge, fill=0.0,
            base=0, pattern=[[1, 128]], channel_multiplier=-1,
        )

        eps_t = const.tile([128, 1], FP)
        nc.vector.memset(eps_t, EPS)

        bq_b = const.tile([128, 1], FP)
        bk_b = const.tile([128, 1], FP)
        bv_b = const.tile([128, 1], FP)
        nc.sync.dma_start(out=bq_b, in_=bq_t.ap())
        nc.sync.dma_start(out=bk_b, in_=bk_t.ap())
        nc.sync.dma_start(out=bv_b, in_=bv_t.ap())

        # ---------- comm bounce buffers ----------
        ag_in = dram.tile([NSTRIP, KT, 128, 128], BF)
        ag_out = dram.tile([NCORES, NSTRIP, KT, 128, 128], BF,
                           addr_space="Shared")
        a2a_in = dram.tile([2, NCORES, 128, 2 * 128], BF)
        a2a_out = [dram.tile([NCORES, 128, 2 * 128], BF,
                             name=f"a2a_out{hh}") for hh in range(2)]

        wq_sb = wpool.tile([128, KT, HPC * DH], BF)
        wk_sb = wpool.tile([128, KT, HPC * DH], BF)
        wv_sb = wpool.tile([128, KT, HPC * DH], BF)
        wo_sb = wpool.tile([128, KT, D], BF)

        # ---------- phase A: per-strip LayerNorm + transpose + AllGather ---
        with tc.tile_pool(name="lnp", bufs=2) as lnp, \
             tc.tile_pool(name="lns", bufs=2) as lns, \
             tc.tile_pool(name="tstage", bufs=2) as tstage:
            nc.sync.dma_start(
                out=wq_sb,
                in_=wq_t.ap().rearrange("(k p) c -> p k c", p=128))
            nc.sync.dma_start(
                out=wk_sb,
                in_=wk_t.ap().rearrange("(k p) c -> p k c", p=128))
            nc.sync.dma_start(
                out=wv_sb,
                in_=wv_t.ap().rearrange("(k p) c -> p k c", p=128))
            x_tiles = []
            for r in range(NSTRIP):
                x_t = lnp.tile([128, D], FP, tag=f"x{r}", name=f"x_{r}",
                               bufs=1)
                nc.sync.dma_start(out=x_t, in_=x_sh.ap()[ts(r, 128), :])
                x_tiles.append(x_t)
            for r in range(NSTRIP):
                x_t = x_tiles[r]
                stats = lns.tile([128, 2, 6], FP, tag="stats")
                xg = x_t.rearrange("p (s f) -> p s f", f=512)
                for s in range(2):
                    nc.vector.bn_stats(out=stats[:, s, :], in_=xg[:, s, :])
                mv = lns.tile([128, 2], FP, tag="mv")
                nc.vector.bn_aggr(out=mv, in_=stats)
                rstd = lns.tile([128, 1], FP, tag="rstd")
                nc.scalar.activation(out=rstd, in_=mv[:, 1:2],
                                     func=mybir.ActivationFunctionType.Sqrt,
                                     bias=eps_t, scale=1.0)
                nc.vector.reciprocal(out=rstd, in_=rstd)
                xn_bf = lnp.tile([128, D], BF, tag="xnbf")
                nc.vector.tensor_scalar(
                    out=xn_bf, in0=x_t, scalar1=mv[:, 0:1], scalar2=rstd,
                    op0=mybir.AluOpType.subtract, op1=mybir.AluOpType.mult)
                xnT_s = tstage.tile([128, KT, 128], BF, tag="xnT")
                nc.scalar.dma_start_transpose(out=xnT_s, in_=xn_bf)
                nc.sync.dma_start(
                    out=ag_in[r].rearrange("k p t -> p k t"),
                    in_=xnT_s)

        nc.gpsimd.collective_compute(
            "AllGather", mybir.AluOpType.bypass,
            replica_groups=[list(range(NCORES))],
            ins=[ag_in.opt()], outs=[ag_out.opt()])

        # ---------- phase B+C: QKV + interleaved attention + A2A/out-proj --
        qT = big.tile([128, TCH, 512], BF)   # rows: [h0 64 | h1 64]
        kTt = big.tile([128, TCH, 512], BF)
        vhat = big.tile([128, JTB * B, 130], BF)  # col 64/129 = 1
        nc.gpsimd.memset(vhat[:, :, 64:65], 1.0)
        nc.gpsimd.memset(vhat[:, :, 129:130], 1.0)
        outT = [big.tile([64, T], BF, name=f"outT{h}") for h in range(HPC)]
        a2a_sb = big.tile([128, NCORES, 512], BF)
        out_view = out_sh.ap().rearrange("(t p) e -> p t e", p=128)

        def a2a_stage(hh):
            # ship outT column strips {1024*(2hh+r') + 128*c} to core c
            for c in range(NCORES):
                for h in range(HPC):
                    src = outT[h].rearrange(
                        "p (r q t) -> p r q t", r=NSTRIP, q=NCORES)
                    nc.sync.dma_start(
                        out=a2a_in[hh, c, ds(64 * h, 64), :].rearrange(
                            "p (r t) -> p r t", r=2),
                        in_=src[:, ds(2 * hh, 2), c, :])
            nc.gpsimd.collective_compute(
                "AllToAll", mybir.AluOpType.bypass,
                replica_groups=[list(range(NCORES))],
                ins=[a2a_in[hh].opt()], outs=[a2a_out[hh].opt()])

        def op_thunks(hh):
            # output projection for token tiles of half hh (strips 2hh,2hh+1)
            thunks = []

            def load():
                for ct in range(NCORES):
                    nc.sync.dma_start(
                        out=a2a_sb[:, ct, ds(256 * hh, 256)],
                        in_=a2a_out[hh][ct])
            thunks.append(load)
            for rr in range(2):
                tt = 2 * hh + rr
                for ec in range(D // 512):
                    po = qkv_ps.tile([128, 512], FP, tag="acc",
                                     name=f"po_{tt}_{ec}")

                    def mms(po=po, tt=tt, ec=ec):
                        for ct in range(NCORES):
                            nc.tensor.matmul(
                                po, a2a_sb[:, ct, ds(128 * tt, 128)],
                                wo_sb[:, ct, ds(512 * ec, 512)],
                                start=(ct == 0), stop=(ct == NCORES - 1))
                    thunks.append(mms)

                    def store(po=po, tt=tt, ec=ec):
                        ost = vst_holder[0].tile([128, 512], FP, tag="ost",
                                                 name=f"ost_{tt}_{ec}")
                        nc.vector.tensor_copy(out=ost, in_=po)
                        nc.sync.dma_start(
                            out=out_view[:, tt, ds(512 * ec, 512)], in_=ost)
                    thunks.append(store)
            return thunks

        vst_holder = [None]
        with tc.tile_pool(name="xstream", bufs=2) as xstream, \
             tc.tile_pool(name="qkv_ps", bufs=2, space="PSUM") as qkv_ps, \
             tc.tile_pool(name="s_ps", bufs=2, space="PSUM") as s_ps, \
             tc.tile_pool(name="av_ps", bufs=1, space="PSUM") as av_ps, \
             tc.tile_pool(name="espool", bufs=6) as espool, \
             tc.tile_pool(name="smallp", bufs=4) as smallp, \
             tc.tile_pool(name="ostp", bufs=3) as ostp, \
             tc.tile_pool(name="vstage", bufs=2) as vst:
            vst_holder[0] = ostp
            pending_att = None
            for r in range(NSTRIP):
                xt = xstream.tile([128, KT, NCORES * 128], BF, tag="xt",
                                  name=f"xt_{r}")
                for c in range(NCORES):
                    nc.sync.dma_start(
                        out=xt[:, :, ds(128 * c, 128)],
                        in_=ag_out[c, r].rearrange("k p t -> p k t"))
                if r == 1:
                    nc.sync.dma_start(
                        out=wo_sb,
                        in_=wo_t.ap().rearrange("(k p) e -> p k e", p=128))
                for u in range(2):
                    tci = 2 * r + u
                    qkv = _qkv_thunks(nc, tci, xt[:, :, ds(512 * u, 512)],
                                      wq_sb, wk_sb, wv_sb, bq_b, bk_b, bv_b,
                                      qT, kTt, vhat, qkv_ps, vst)
                    if pending_att is None:
                        seq = qkv
                    else:
                        seq = _merge(pending_att, qkv)
                    for thunk in seq:
                        thunk()
                    if tci == 4:
                        # batch-0 attention (chunks 0-3) fully emitted
                        a2a_stage(0)
                    b_, ic = tci // ICB, tci % ICB
                    pending_att = _att_thunks(nc, b_, ic, kTt, qT, vhat, tri,
                                              outT, s_ps, av_ps, espool,
                                              smallp)
            # last chunk's attention, with first-half out-proj interleaved
            seq = _merge(pending_att, op_thunks(0))
            for thunk in seq:
                thunk()
            a2a_stage(1)
            for thunk in op_thunks(1):
                thunk()


_NC = None
LAST_EXEC_TIME_NS = None


def _get_nc():
    global _NC
    if _NC is None:
        _NC = build()
    return _NC


def make_in_maps(x, gamma, beta, w_qkv, w_out):
    bf = ml_dtypes.bfloat16
    x = np.ascontiguousarray(np.asarray(x, dtype=np.float32)).reshape(T, D)
    gamma = np.asarray(gamma, dtype=np.float32)
    beta = np.asarray(beta, dtype=np.float32)
    w_qkv = np.asarray(w_qkv, dtype=np.float32)
    qkv_bias = beta @ w_qkv                      # [3*INNER]
    w_qkv = (w_qkv * gamma[:, None]).astype(bf)  # fold gamma into weights
    w_out = np.ascontiguousarray(np.asarray(w_out, dtype=np.float32).astype(bf))
    xs = x.reshape(NSTRIP, NCORES, 128, D)
    in_maps = []
    for c in range(NCORES):
        cols = slice(128 * c, 128 * c + 128)
        in_maps.append({
            "x_shard": np.ascontiguousarray(xs[:, c].reshape(TS, D)),
            "wq": np.ascontiguousarray(w_qkv[:, cols]),
            "wk": np.ascontiguousarray(w_qkv[:, INNER:][:, cols]),
            "wv": np.ascontiguousarray(w_qkv[:, 2 * INNER:][:, cols]),
            "bq": np.ascontiguousarray(qkv_bias[cols].reshape(128, 1)),
            "bk": np.ascontiguousarray(qkv_bias[INNER:][cols].reshape(128, 1)),
            "bv": np.ascontiguousarray(
                qkv_bias[2 * INNER:][cols].reshape(128, 1)),
            "w_out": w_out,
        })
    return in_maps


def kernel(x, mask, gamma, beta, w_qkv, w_out):
    global LAST_EXEC_TIME_NS
    nc = _get_nc()
    in_maps = make_in_maps(x, gamma, beta, w_qkv, w_out)
    res = run_bass_kernel_spmd(nc, in_maps, core_ids=list(range(NCORES)))
    LAST_EXEC_TIME_NS = res.exec_time_ns
    out = np.zeros((NSTRIP, NCORES, 128, D), dtype=np.float32)
    for c in range(NCORES):
        out[:, c] = res.results[c]["out_shard"].reshape(NSTRIP, 128, D)
    return out.reshape(B, N, D).astype(np.float32)


# revision 19
# speedup vs baseline: 1.0736x; 1.0736x over previous
"""Distributed Trainium2 kernel: LayerNorm -> QKV -> causal MHA -> out-proj.

Sharding (8 cores):
  - LayerNorm + final projection: token-parallel, strip-interleaved: core c
    owns token strips {1024*r + 128*c + [0,128) : r=0..3} (4096 tokens ->
    512/core).  Strip interleaving lets the post-LN AllGather be split into
    4 pipelined 2MiB collectives, each gathering one contiguous 1024-token
    block, so QKV/attention compute starts after the first lands.
  - Attention + QKV: head-parallel (16 heads -> 2/core).
  - Comms: 4x AllGather of xn^T strips (bf16) after LN; 2x AllToAll of
    per-head attention output (one per 2048-token half) re-shards the
    inner-dim contraction to token-parallel; no AllReduce needed.

Layout notes:
  - All activations are kept TRANSPOSED ([feature, token]) so every matmul
    contraction runs over the partition axis.  S is computed transposed
    (S^T[j,i] = k_j . q_i); softmax sums come from an appended ones-column
    on V (m=65 matmul); the dh^-0.5 scale rides the exp activation's free
    affine.  Streams are causally trimmed at 128 granularity; the single
    partial diagonal 128x128 tile is zeroed with a triangle mask post-exp.
  - The two heads' K=64 S-matmuls are emitted back-to-back at row-disjoint
    tile_position (0,0)/(64,0) so they co-execute on the PE array.
  - gamma/beta are folded host-side into w_qkv (row scale) and per-feature
    biases (beta @ w_qkv), so LN on-device is just (x-mu)*rstd.
  - Matmul inputs are bf16 (4x the fp32 TensorE rate); accumulation fp32.
  - Attention groups are interleaved into the QKV token-chunk loop so
    ScalarE exp work overlaps TensorE QKV matmuls and the PE stream stays
    dense (HAM-warm).  The first AllToAll + its out-projection are
    interleaved into the attention tail.
"""

import numpy as np
import ml_dtypes

import concourse.bass as bass
import concourse.tile as tile
from concourse import bacc, mybir
from concourse.bass import ds, ts
from concourse.bass_utils import run_bass_kernel_spmd
from concourse.masks import make_identity

B, N, D = 2, 2048, 1024
HEADS, DH = 16, 64
INNER = HEADS * DH          # 1024
NCORES = 8
T = B * N                   # 4096 tokens
TS = T // NCORES            # 512 tokens per core
HPC = HEADS // NCORES       # 2 heads per core
SCALE = float(DH) ** -0.5   # 0.125
EPS = 1e-5

FP = mybir.dt.float32
BF = mybir.dt.bfloat16

KT = D // 128               # 8 contraction tiles of 128 over dim
TCH = T // 512              # 8 token chunks of 512
ICB = N // 512              # 4 i-chunks of 512 per batch
JTB = N // 128              # 16 j-tiles of 128 per batch
NSTRIP = 4                  # AllGather rounds; round r = tokens [1024r,1024r+1024)


def build():
    nc = bacc.Bacc("TRN2", target_bir_lowering=False, debug=False,
                   num_devices=NCORES)

    x_sh = nc.dram_tensor("x_shard", [TS, D], FP, kind="ExternalInput")
    wq_t = nc.dram_tensor("wq", [D, HPC * DH], BF, kind="ExternalInput")
    wk_t = nc.dram_tensor("wk", [D, HPC * DH], BF, kind="ExternalInput")
    wv_t = nc.dram_tensor("wv", [D, HPC * DH], BF, kind="ExternalInput")
    bq_t = nc.dram_tensor("bq", [HPC * DH, 1], FP, kind="ExternalInput")
    bk_t = nc.dram_tensor("bk", [HPC * DH, 1], FP, kind="ExternalInput")
    bv_t = nc.dram_tensor("bv", [HPC * DH, 1], FP, kind="ExternalInput")
    wo_t = nc.dram_tensor("w_out", [INNER, D], BF, kind="ExternalInput")
    out_sh = nc.dram_tensor("out_shard", [TS, D], FP, kind="ExternalOutput")

    with tile.TileContext(nc) as tc:
        _body(nc, tc, x_sh, wq_t, wk_t, wv_t, bq_t, bk_t, bv_t, wo_t, out_sh)

    nc.compile()
    return nc


def _att_thunks(nc, b, ic, kTt, qT, vhat, tri, outT,
                s_ps, av_ps, espool, smallp):
    """Attention for query chunk (b, ic) as a list of PE-ordered thunks.

    Each jp step emits the two heads' S matmuls in a single thunk pair so
    the row-disjoint (tile_position 0 / 64) K=64 matmuls stay adjacent in
    the PE queue and co-execute.  S-matmuls for step jp are emitted before
    the AV-matmuls of step jp-1 so the PE never waits on ACT exp latency.
    """
    q_idx = b * ICB + ic
    njt = 4 * (ic + 1)
    av = [av_ps.tile([128, 512], FP, tag=f"av{h}", name=f"av{h}_{q_idx}")
          for h in range(HPC)]
    es = {}

    def i0_of(jt):
        m = jt - 4 * ic
        return 128 * m if m > 0 else 0

    def s_pair(jp, u):
        # Both heads' S^T for j-tile jt land in ONE PSUM tile (h0 cols
        # 0-511, h1 cols 512-1023) so a single exp releases the slot and
        # the scheduler cannot split the co-executing pair.
        def run():
            sx = s_ps.tile([128, 1024], FP, tag="sx",
                           name=f"sx_{q_idx}_{jp}_{u}")
            es[(jp, u)] = (sx, None)
            jt = 2 * jp + u
            tq = b * ICB + jt // 4
            jo = 128 * (jt % 4)
            i0 = i0_of(jt)
            for h in range(HPC):
                nc.tensor.matmul(
                    sx[:, ds(512 * h + i0, 512 - i0)],
                    kTt[ds(64 * h, 64), tq, ds(jo, 128)],
                    qT[ds(64 * h, 64), q_idx, ds(i0, 512 - i0)],
                    start=True, stop=True,
                    tile_position=(64 * h, 0))
        return run

    def exp_t(jp, u):
        def run():
            sx, _ = es[(jp, u)]
            e = espool.tile([128, 1024], BF, tag="es",
                            name=f"es_{q_idx}_{jp}_{u}")
            es[(jp, u)] = (sx, e)
            i0 = i0_of(2 * jp + u)
            if i0 == 0:
                nc.scalar.activation(
                    out=e, in_=sx,
                    func=mybir.ActivationFunctionType.Exp, scale=SCALE)
            else:
                nc.scalar.activation(
                    out=e.rearrange("p (h i) -> p h i", h=2)[:, :, i0:],
                    in_=sx.rearrange("p (h i) -> p h i", h=2)[:, :, i0:],
                    func=mybir.ActivationFunctionType.Exp, scale=SCALE)
        return run

    def av_mm(h, jp, u):
        def run():
            _, e = es[(jp, u)]
            jt = 2 * jp + u
            m = jt - 4 * ic
            i0 = i0_of(jt)
            if 0 <= m < 4:
                # zero the strict upper triangle of the diagonal 128x128 tile
                nc.vector.tensor_tensor(
                    out=e[:, ds(512 * h + 128 * m, 128)],
                    in0=e[:, ds(512 * h + 128 * m, 128)],
                    in1=tri,
                    op=mybir.AluOpType.mult)
            nc.tensor.matmul(
                av[h][0:65, ds(i0, 512 - i0)],
                vhat[:, b * JTB + jt, ds(65 * h, 65)],
                e[:, ds(512 * h + i0, 512 - i0)],
                start=(jt == 0), stop=(jt == njt - 1))
        return run

    def norm(h):
        def run():
            rsum = smallp.tile([1, 512], FP, tag="rsum", name=f"rs{h}_{q_idx}")
            nc.vector.tensor_copy(out=rsum, in_=av[h][64:65, :])
            rec = smallp.tile([1, 512], FP, tag="rec", name=f"rc{h}_{q_idx}")
            nc.vector.reciprocal_approx_fast(out=rec, in_=rsum)
            bc = smallp.tile([64, 512], FP, tag="bc", name=f"bc{h}_{q_idx}")
            nc.gpsimd.partition_broadcast(bc, rec)
            nc.vector.tensor_tensor(
                out=outT[h][:, ds(512 * q_idx, 512)],
                in0=av[h][0:64, :], in1=bc,
                op=mybir.AluOpType.mult)
        return run

    def s_group(jp):
        return [s_pair(jp, 0), s_pair(jp, 1), exp_t(jp, 0), exp_t(jp, 1)]

    def av_group(jp):
        return [av_mm(0, jp, 0), av_mm(0, jp, 1),
                av_mm(1, jp, 0), av_mm(1, jp, 1)]

    thunks = []
    nps = njt // 2
    thunks.extend(s_group(0))
    for jp in range(1, nps):
        thunks.extend(s_group(jp))
        thunks.extend(av_group(jp - 1))
    thunks.extend(av_group(nps - 1))
    thunks.append(norm(0))
    thunks.append(norm(1))
    return thunks


def _qkv_thunks(nc, tci, xt_u, wq_sb, wk_sb, wv_sb, bq_b, bk_b, bv_b,
                qT, kTt, vhat, qkv_ps, vst):
    """QKV projection for token chunk tci as a list of PE-ordered thunks."""
    thunks = []

    def mk_group(w_sb, nm):
        acc = qkv_ps.tile([128, 512], FP, tag="acc", name=f"acc{nm}_{tci}")

        def mm(k):
            def run():
                nc.tensor.matmul(acc, w_sb[:, k, :], xt_u[:, k, :],
                                 start=(k == 0), stop=(k == KT - 1))
            return run
        return acc, mm

    for w_sb, bias, dst, nm in ((wq_sb, bq_b, qT, "q"),
                                (wk_sb, bk_b, kTt, "k")):
        acc, mm = mk_group(w_sb, nm)
        for k in range(KT):
            thunks.append(mm(k))

        def copy(acc=acc, bias=bias, dst=dst, tci=tci):
            nc.vector.tensor_scalar(
                out=dst[:, tci, :], in0=acc, scalar1=bias, scalar2=None,
                op0=mybir.AluOpType.add)
        thunks.append(copy)

    accv, mmv = mk_group(wv_sb, "v")
    for k in range(KT):
        thunks.append(mmv(k))
    vs = vst.tile([128, 512], BF, tag="vs", name=f"vs_{tci}")

    def vcopy():
        nc.vector.tensor_scalar(
            out=vs, in0=accv, scalar1=bv_b, scalar2=None,
            op0=mybir.AluOpType.add)
    thunks.append(vcopy)

    def vtrans():
        def run():
            vstg = vst.tile([128, 4, 128], BF, tag="vstg", name=f"vstg_{tci}")
            nc.sync.dma_start_transpose(out=vstg, in_=vs)
            nc.vector.tensor_copy(out=vhat[:, ds(tci * 4, 4), 0:64],
                                  in_=vstg[:, :, 0:64])
            nc.vector.tensor_copy(out=vhat[:, ds(tci * 4, 4), 65:129],
                                  in_=vstg[:, :, 64:128])
        return run
    thunks.append(vtrans())
    return thunks


def _merge(primary, filler):
    """Interleave filler thunks evenly between primary thunks."""
    out = []
    np_, nf = len(primary), len(filler)
    fi = 0
    for i, p in enumerate(primary):
        out.append(p)
        want = (i + 1) * nf // np_
        while fi < want:
            out.append(filler[fi])
            fi += 1
    out.extend(filler[fi:])
    return out


def _body(nc, tc, x_sh, wq_t, wk_t, wv_t, bq_t, bk_t, bv_t, wo_t, out_sh):
    from contextlib import ExitStack
    ctx = ExitStack()
    with ctx:
        const = ctx.enter_context(tc.tile_pool(name="const", bufs=1))
        wpool = ctx.enter_context(tc.tile_pool(name="wpool", bufs=1))
        big = ctx.enter_context(tc.tile_pool(name="big", bufs=1))
        dram = ctx.enter_context(tc.tile_pool(name="dram", bufs=1, space="DRAM"))

        # ---------- constants ----------
        identity = const.tile([128, 128], BF)
        make_identity(nc, identity)

        # 0/1 lower-triangle mask for the one partial 128x128 diagonal tile
        # of S^T: keep [jj, ii] iff ii - jj >= 0.
        tri = const.tile([128, 128], BF)
        nc.gpsimd.memset(tri, 1.0)
        nc.gpsimd.affine_select(
            out=tri, in_=tri,
            compare_op=mybir.AluOpType.is_ge, fill=0.0,
            base=0, pattern=[[1, 128]], channel_multiplier=-1,
        )

        eps_t = const.tile([128, 1], FP)
        nc.vector.memset(eps_t, EPS)

        bq_b = const.tile([128, 1], FP)
        bk_b = const.tile([128, 1], FP)
        bv_b = const.tile([128, 1], FP)
        nc.sync.dma_start(out=bq_b, in_=bq_t.ap())
        nc.sync.dma_start(out=bk_b, in_=bk_t.ap())
        nc.sync.dma_start(out=bv_b, in_=bv_t.ap())

        # ---------- comm bounce buffers ----------
        ag_in = dram.tile([NSTRIP, KT, 128, 128], BF)
        ag_out = dram.tile([NCORES, NSTRIP, KT, 128, 128], BF,
                           addr_space="Shared")
        a2a_in = dram.tile([2, NCORES, 128, 2 * 128], BF)
        a2a_out = [dram.tile([NCORES, 128, 2 * 128], BF,
                             name=f"a2a_out{hh}") for hh in range(2)]

        wq_sb = wpool.tile([128, KT, HPC * DH], BF)
        wk_sb = wpool.tile([128, KT, HPC * DH], BF)
        wv_sb = wpool.tile([128, KT, HPC * DH], BF)
        wo_sb = wpool.tile([128, KT, D], BF)

        # ---------- phase A: per-strip LayerNorm + transpose + AllGather ---
        with tc.tile_pool(name="lnp", bufs=2) as lnp, \
             tc.tile_pool(name="lns", bufs=2) as lns, \
             tc.tile_pool(name="tstage", bufs=2) as tstage:
            nc.sync.dma_start(
                out=wq_sb,
                in_=wq_t.ap().rearrange("(k p) c -> p k c", p=128))
            nc.sync.dma_start(
                out=wk_sb,
                in_=wk_t.ap().rearrange("(k p) c -> p k c", p=128))
            nc.sync.dma_start(
                out=wv_sb,
                in_=wv_t.ap().rearrange("(k p) c -> p k c", p=128))
            x_tiles = []
            for r in range(NSTRIP):
                x_t = lnp.tile([128, D], FP, tag=f"x{r}", name=f"x_{r}",
                               bufs=1)
                nc.sync.dma_start(out=x_t, in_=x_sh.ap()[ts(r, 128), :])
                x_tiles.append(x_t)
            for r in range(NSTRIP):
                x_t = x_tiles[r]
                stats = lns.tile([128, 2, 6], FP, tag="stats")
                xg = x_t.rearrange("p (s f) -> p s f", f=512)
                for s in range(2):
                    nc.vector.bn_stats(out=stats[:, s, :], in_=xg[:, s, :])
                mv = lns.tile([128, 2], FP, tag="mv")
                nc.vector.bn_aggr(out=mv, in_=stats)
                rstd = lns.tile([128, 1], FP, tag="rstd")
                nc.scalar.activation(out=rstd, in_=mv[:, 1:2],
                                     func=mybir.ActivationFunctionType.Sqrt,
                                     bias=eps_t, scale=1.0)
                nc.vector.reciprocal(out=rstd, in_=rstd)
                xn_bf = lnp.tile([128, D], BF, tag="xnbf")
                nc.vector.tensor_scalar(
                    out=xn_bf, in0=x_t, scalar1=mv[:, 0:1], scalar2=rstd,
                    op0=mybir.AluOpType.subtract, op1=mybir.AluOpType.mult)
                xnT_s = tstage.tile([128, KT, 128], BF, tag="xnT")
                nc.scalar.dma_start_transpose(out=xnT_s, in_=xn_bf)
                nc.sync.dma_start(
                    out=ag_in[r].rearrange("k p t -> p k t"),
                    in_=xnT_s)

        nc.gpsimd.collective_compute(
            "AllGather", mybir.AluOpType.bypass,
            replica_groups=[list(range(NCORES))],
            ins=[ag_in.opt()], outs=[ag_out.opt()])

        # ---------- phase B+C: QKV + interleaved attention + A2A/out-proj --
        qT = big.tile([128, TCH, 512], BF)   # rows: [h0 64 | h1 64]
        kTt = big.tile([128, TCH, 512], BF)
        vhat = big.tile([128, JTB * B, 130], BF)  # col 64/129 = 1
        nc.gpsimd.memset(vhat[:, :, 64:65], 1.0)
        nc.gpsimd.memset(vhat[:, :, 129:130], 1.0)
        outT = [big.tile([64, T], BF, name=f"outT{h}") for h in range(HPC)]
        a2a_sb = big.tile([128, NCORES, 512], BF)
        out_view = out_sh.ap().rearrange("(t p) e -> p t e", p=128)

        def a2a_stage(hh):
            # ship outT column strips {1024*(2hh+r') + 128*c} to core c
            for c in range(NCORES):
                for h in range(HPC):
                    src = outT[h].rearrange(
                        "p (r q t) -> p r q t", r=NSTRIP, q=NCORES)
                    nc.sync.dma_start(
                        out=a2a_in[hh, c, ds(64 * h, 64), :].rearrange(
                            "p (r t) -> p r t", r=2),
                        in_=src[:, ds(2 * hh, 2), c, :])
            nc.gpsimd.collective_compute(
                "AllToAll", mybir.AluOpType.bypass,
                replica_groups=[list(range(NCORES))],
                ins=[a2a_in[hh].opt()], outs=[a2a_out[hh].opt()])

        def op_thunks(hh):
            # output projection for token tiles of half hh (strips 2hh,2hh+1)
            thunks = []

            def load():
                for ct in range(NCORES):
                    nc.sync.dma_start(
                        out=a2a_sb[:, ct, ds(256 * hh, 256)],
                        in_=a2a_out[hh][ct])
            thunks.append(load)
            for rr in range(2):
                tt = 2 * hh + rr
                for ec in range(D // 512):
                    po = qkv_ps.tile([128, 512], FP, tag="acc",
                                     name=f"po_{tt}_{ec}")

                    def mms(po=po, tt=tt, ec=ec):
                        for ct in range(NCORES):
                            nc.tensor.matmul(
                                po, a2a_sb[:, ct, ds(128 * tt, 128)],
                                wo_sb[:, ct, ds(512 * ec, 512)],
                                start=(ct == 0), stop=(ct == NCORES - 1))
                    thunks.append(mms)

                    def store(po=po, tt=tt, ec=ec):
                        ost = vst_holder[0].tile([128, 512], FP, tag="ost",
                                                 name=f"ost_{tt}_{ec}")
                        nc.vector.tensor_copy(out=ost, in_=po)
                        nc.sync.dma_start(
                            out=out_view[:, tt, ds(512 * ec, 512)], in_=ost)
                    thunks.append(store)
            return thunks

        vst_holder = [None]
        with tc.tile_pool(name="xstream", bufs=2) as xstream, \
             tc.tile_pool(name="qkv_ps", bufs=2, space="PSUM") as qkv_ps, \
             tc.tile_pool(name="s_ps", bufs=2, space="PSUM") as s_ps, \
             tc.tile_pool(name="av_ps", bufs=1, space="PSUM") as av_ps, \
             tc.tile_pool(name="espool", bufs=6) as espool, \
             tc.tile_pool(name="smallp", bufs=4) as smallp, \
             tc.tile_pool(name="ostp", bufs=3) as ostp, \
             tc.tile_pool(name="vstage", bufs=2) as vst:
            vst_holder[0] = ostp
            pending_att = None
            for r in range(NSTRIP):
                xt = xstream.tile([128, KT, NCORES * 128], BF, tag="xt",
                                  name=f"xt_{r}")
                for c in range(NCORES):
                    nc.sync.dma_start(
                        out=xt[:, :, ds(128 * c, 128)],
                        in_=ag_out[c, r].rearrange("k p t -> p k t"))
                if r == 1:
                    nc.sync.dma_start(
                        out=wo_sb,
                        in_=wo_t.ap().rearrange("(k p) e -> p k e", p=128))
                for u in range(2):
                    tci = 2 * r + u
                    qkv = _qkv_thunks(nc, tci, xt[:, :, ds(512 * u, 512)],
                                      wq_sb, wk_sb, wv_sb, bq_b, bk_b, bv_b,
                                      qT, kTt, vhat, qkv_ps, vst)
                    if pending_att is None:
                        seq = qkv
                    else:
                        seq = _merge(pending_att, qkv)
                    for thunk in seq:
                        thunk()
                    if tci == 4:
                        # batch-0 attention (chunks 0-3) fully emitted
                        a2a_stage(0)
                    b_, ic = tci // ICB, tci % ICB
                    pending_att = _att_thunks(nc, b_, ic, kTt, qT, vhat, tri,
                                              outT, s_ps, av_ps, espool,
                                              smallp)
            # last chunk's attention, then first-half out-proj (appended, not
            # merged: a merge would let a not-yet-ready op matmul stall the
            # strict-FIFO PE queue behind it)
            for thunk in pending_att:
                thunk()
            for thunk in op_thunks(0):
                thunk()
            a2a_stage(1)
            for thunk in op_thunks(1):
                thunk()


_NC = None
LAST_EXEC_TIME_NS = None


def _get_nc():
    global _NC
    if _NC is None:
        _NC = build()
    return _NC


def make_in_maps(x, gamma, beta, w_qkv, w_out):
    bf = ml_dtypes.bfloat16
    x = np.ascontiguousarray(np.asarray(x, dtype=np.float32)).reshape(T, D)
    gamma = np.asarray(gamma, dtype=np.float32)
    beta = np.asarray(beta, dtype=np.float32)
    w_qkv = np.asarray(w_qkv, dtype=np.float32)
    qkv_bias = beta @ w_qkv                      # [3*INNER]
    w_qkv = (w_qkv * gamma[:, None]).astype(bf)  # fold gamma into weights
    w_out = np.ascontiguousarray(np.asarray(w_out, dtype=np.float32).astype(bf))
    xs = x.reshape(NSTRIP, NCORES, 128, D)
    in_maps = []
    for c in range(NCORES):
        cols = slice(128 * c, 128 * c + 128)
        in_maps.append({
            "x_shard": np.ascontiguousarray(xs[:, c].reshape(TS, D)),
            "wq": np.ascontiguousarray(w_qkv[:, cols]),
            "wk": np.ascontiguousarray(w_qkv[:, INNER:][:, cols]),
            "wv": np.ascontiguousarray(w_qkv[:, 2 * INNER:][:, cols]),
            "bq": np.ascontiguousarray(qkv_bias[cols].reshape(128, 1)),
            "bk": np.ascontiguousarray(qkv_bias[INNER:][cols].reshape(128, 1)),
            "bv": np.ascontiguousarray(
                qkv_bias[2 * INNER:][cols].reshape(128, 1)),
            "w_out": w_out,
        })
    return in_maps


def kernel(x, mask, gamma, beta, w_qkv, w_out):
    global LAST_EXEC_TIME_NS
    nc = _get_nc()
    in_maps = make_in_maps(x, gamma, beta, w_qkv, w_out)
    res = run_bass_kernel_spmd(nc, in_maps, core_ids=list(range(NCORES)))
    LAST_EXEC_TIME_NS = res.exec_time_ns
    out = np.zeros((NSTRIP, NCORES, 128, D), dtype=np.float32)
    for c in range(NCORES):
        out[:, c] = res.results[c]["out_shard"].reshape(NSTRIP, 128, D)
    return out.reshape(B, N, D).astype(np.float32)


# revision 25
# speedup vs baseline: 1.1067x; 1.0308x over previous
"""Distributed Trainium2 kernel: LayerNorm -> QKV -> causal MHA -> out-proj.

Sharding (8 cores):
  - LayerNorm + final projection: token-parallel, strip-interleaved: core c
    owns token strips {1024*r + 128*c + [0,128) : r=0..3} (4096 tokens ->
    512/core).  Strip interleaving lets the post-LN AllGather be split into
    4 pipelined 2MiB collectives, each gathering one contiguous 1024-token
    block, so QKV/attention compute starts after the first lands.
  - Attention + QKV: head-parallel (16 heads -> 2/core).
  - Comms: 4x AllGather of xn^T strips (bf16) after LN; 2x AllToAll of
    per-head attention output (one per 2048-token half) re-shards the
    inner-dim contraction to token-parallel; no AllReduce needed.

Layout notes:
  - All activations are kept TRANSPOSED ([feature, token]) so every matmul
    contraction runs over the partition axis.  S is computed transposed
    (S^T[j,i] = k_j . q_i); softmax sums come from an appended ones-column
    on V (m=65 matmul); the dh^-0.5 scale rides the exp activation's free
    affine.  Streams are causally trimmed at 128 granularity; the single
    partial diagonal 128x128 tile is zeroed with a triangle mask post-exp.
  - The two heads' K=64 S-matmuls are emitted back-to-back at row-disjoint
    tile_position (0,0)/(64,0) so they co-execute on the PE array.
  - gamma/beta are folded host-side into w_qkv (row scale) and per-feature
    biases (beta @ w_qkv), so LN on-device is just (x-mu)*rstd.
  - Matmul inputs are bf16 (4x the fp32 TensorE rate); accumulation fp32.
  - Attention groups are interleaved into the QKV token-chunk loop so
    ScalarE exp work overlaps TensorE QKV matmuls and the PE stream stays
    dense (HAM-warm).  The first AllToAll + its out-projection are
    interleaved into the attention tail.
"""

import numpy as np
import ml_dtypes

import concourse.bass as bass
import concourse.tile as tile
from concourse import bacc, mybir
from concourse.bass import ds, ts
from concourse.bass_utils import run_bass_kernel_spmd
from concourse.masks import make_identity

B, N, D = 2, 2048, 1024
HEADS, DH = 16, 64
INNER = HEADS * DH          # 1024
NCORES = 8
T = B * N                   # 4096 tokens
TS = T // NCORES            # 512 tokens per core
HPC = HEADS // NCORES       # 2 heads per core
SCALE = float(DH) ** -0.5   # 0.125
EPS = 1e-5

FP = mybir.dt.float32
BF = mybir.dt.bfloat16

KT = D // 128               # 8 contraction tiles of 128 over dim
TCH = T // 512              # 8 token chunks of 512
ICB = N // 512              # 4 i-chunks of 512 per batch
JTB = N // 128              # 16 j-tiles of 128 per batch
NSTRIP = 4                  # AllGather rounds; round r = tokens [1024r,1024r+1024)


def build():
    nc = bacc.Bacc("TRN2", target_bir_lowering=False, debug=False,
                   num_devices=NCORES)

    x_sh = nc.dram_tensor("x_shard", [TS, D], FP, kind="ExternalInput")
    wq_t = nc.dram_tensor("wq", [D, HPC * DH], BF, kind="ExternalInput")
    wk_t = nc.dram_tensor("wk", [D, HPC * DH], BF, kind="ExternalInput")
    wv_t = nc.dram_tensor("wv", [D, HPC * DH], BF, kind="ExternalInput")
    bq_t = nc.dram_tensor("bq", [HPC * DH, 1], FP, kind="ExternalInput")
    bk_t = nc.dram_tensor("bk", [HPC * DH, 1], FP, kind="ExternalInput")
    bv_t = nc.dram_tensor("bv", [HPC * DH, 1], FP, kind="ExternalInput")
    wo_t = nc.dram_tensor("w_out", [INNER, D], BF, kind="ExternalInput")
    out_sh = nc.dram_tensor("out_shard", [TS, D], FP, kind="ExternalOutput")

    with tile.TileContext(nc) as tc:
        _body(nc, tc, x_sh, wq_t, wk_t, wv_t, bq_t, bk_t, bv_t, wo_t, out_sh)

    nc.compile()
    return nc


def _att_thunks(nc, b, ic, kTt, qT, vhat, tri, outT,
                s_ps, av_ps, espool, smallp):
    """Attention for query chunk (b, ic) as a list of PE-ordered thunks.

    Each jp step emits the two heads' S matmuls in a single thunk pair so
    the row-disjoint (tile_position 0 / 64) K=64 matmuls stay adjacent in
    the PE queue and co-execute.  S-matmuls for step jp are emitted before
    the AV-matmuls of step jp-1 so the PE never waits on ACT exp latency.
    """
    q_idx = b * ICB + ic
    njt = 4 * (ic + 1)
    av = [av_ps.tile([128, 512], FP, tag=f"av{h}", name=f"av{h}_{q_idx}")
          for h in range(HPC)]
    es = {}

    def i0_of(jt):
        m = jt - 4 * ic
        return 128 * m if m > 0 else 0

    def s_pair(jp, u):
        # Both heads' S^T for j-tile jt land in ONE PSUM tile (h0 cols
        # 0-511, h1 cols 512-1023) so a single exp releases the slot and
        # the scheduler cannot split the co-executing pair.
        def run():
            sx = s_ps.tile([128, 1024], FP, tag="sx",
                           name=f"sx_{q_idx}_{jp}_{u}")
            es[(jp, u)] = (sx, None)
            jt = 2 * jp + u
            tq = b * ICB + jt // 4
            jo = 128 * (jt % 4)
            i0 = i0_of(jt)
            for h in range(HPC):
                nc.tensor.matmul(
                    sx[:, ds(512 * h + i0, 512 - i0)],
                    kTt[ds(64 * h, 64), tq, ds(jo, 128)],
                    qT[ds(64 * h, 64), q_idx, ds(i0, 512 - i0)],
                    start=True, stop=True,
                    tile_position=(64 * h, 0))
        return run

    def exp_t(jp, u):
        def run():
            sx, _ = es[(jp, u)]
            e = espool.tile([128, 1024], BF, tag="es",
                            name=f"es_{q_idx}_{jp}_{u}")
            es[(jp, u)] = (sx, e)
            i0 = i0_of(2 * jp + u)
            if i0 == 0:
                nc.scalar.activation(
                    out=e, in_=sx,
                    func=mybir.ActivationFunctionType.Exp, scale=SCALE)
            else:
                nc.scalar.activation(
                    out=e.rearrange("p (h i) -> p h i", h=2)[:, :, i0:],
                    in_=sx.rearrange("p (h i) -> p h i", h=2)[:, :, i0:],
                    func=mybir.ActivationFunctionType.Exp, scale=SCALE)
        return run

    def av_mm(h, jp, u):
        def run():
            _, e = es[(jp, u)]
            jt = 2 * jp + u
            m = jt - 4 * ic
            i0 = i0_of(jt)
            if 0 <= m < 4:
                # zero the strict upper triangle of the diagonal 128x128 tile
                nc.vector.tensor_tensor(
                    out=e[:, ds(512 * h + 128 * m, 128)],
                    in0=e[:, ds(512 * h + 128 * m, 128)],
                    in1=tri,
                    op=mybir.AluOpType.mult)
            nc.tensor.matmul(
                av[h][0:65, ds(i0, 512 - i0)],
                vhat[:, b * JTB + jt, ds(65 * h, 65)],
                e[:, ds(512 * h + i0, 512 - i0)],
                start=(jt == 0), stop=(jt == njt - 1))
        return run

    def norm(h):
        def run():
            rsum = smallp.tile([1, 512], FP, tag="rsum", name=f"rs{h}_{q_idx}")
            nc.vector.tensor_copy(out=rsum, in_=av[h][64:65, :])
            rec = smallp.tile([1, 512], FP, tag="rec", name=f"rc{h}_{q_idx}")
            nc.vector.reciprocal_approx_fast(out=rec, in_=rsum)
            bc = smallp.tile([64, 512], FP, tag="bc", name=f"bc{h}_{q_idx}")
            nc.gpsimd.partition_broadcast(bc, rec)
            nc.vector.tensor_tensor(
                out=outT[h][:, ds(512 * q_idx, 512)],
                in0=av[h][0:64, :], in1=bc,
                op=mybir.AluOpType.mult)
        return run

    def s_group(jp):
        return [s_pair(jp, 0), s_pair(jp, 1), exp_t(jp, 0), exp_t(jp, 1)]

    def av_group(jp):
        return [av_mm(0, jp, 0), av_mm(0, jp, 1),
                av_mm(1, jp, 0), av_mm(1, jp, 1)]

    thunks = []
    nps = njt // 2
    thunks.extend(s_group(0))
    for jp in range(1, nps):
        thunks.extend(s_group(jp))
        thunks.extend(av_group(jp - 1))
    thunks.extend(av_group(nps - 1))
    thunks.append(norm(0))
    thunks.append(norm(1))
    return thunks


def _qkv_thunks(nc, tci, xt_u, wq_sb, wk_sb, wv_sb, bq_b, bk_b, bv_b,
                qT, kTt, vhat, qkv_ps, vst):
    """QKV projection for token chunk tci as a list of PE-ordered thunks."""
    thunks = []

    def mk_group(w_sb, nm):
        acc = qkv_ps.tile([128, 512], FP, tag="acc", name=f"acc{nm}_{tci}")

        def mm(k):
            def run():
                nc.tensor.matmul(acc, w_sb[:, k, :], xt_u[:, k, :],
                                 start=(k == 0), stop=(k == KT - 1))
            return run
        return acc, mm

    for w_sb, bias, dst, nm in ((wq_sb, bq_b, qT, "q"),
                                (wk_sb, bk_b, kTt, "k")):
        acc, mm = mk_group(w_sb, nm)
        for k in range(KT):
            thunks.append(mm(k))

        def copy(acc=acc, bias=bias, dst=dst, tci=tci):
            nc.vector.tensor_scalar(
                out=dst[:, tci, :], in0=acc, scalar1=bias, scalar2=None,
                op0=mybir.AluOpType.add)
        thunks.append(copy)

    accv, mmv = mk_group(wv_sb, "v")
    for k in range(KT):
        thunks.append(mmv(k))
    vs = vst.tile([128, 512], BF, tag="vs", name=f"vs_{tci}")

    def vcopy():
        nc.vector.tensor_scalar(
            out=vs, in0=accv, scalar1=bv_b, scalar2=None,
            op0=mybir.AluOpType.add)
    thunks.append(vcopy)

    def vtrans():
        def run():
            vstg = vst.tile([128, 4, 128], BF, tag="vstg", name=f"vstg_{tci}")
            nc.sync.dma_start_transpose(out=vstg, in_=vs)
            nc.vector.tensor_copy(out=vhat[:, ds(tci * 4, 4), 0:64],
                                  in_=vstg[:, :, 0:64])
            nc.vector.tensor_copy(out=vhat[:, ds(tci * 4, 4), 65:129],
                                  in_=vstg[:, :, 64:128])
        return run
    thunks.append(vtrans())
    return thunks


def _merge(primary, filler):
    """Interleave filler thunks evenly between primary thunks."""
    out = []
    np_, nf = len(primary), len(filler)
    fi = 0
    for i, p in enumerate(primary):
        out.append(p)
        want = (i + 1) * nf // np_
        while fi < want:
            out.append(filler[fi])
            fi += 1
    out.extend(filler[fi:])
    return out


def _body(nc, tc, x_sh, wq_t, wk_t, wv_t, bq_t, bk_t, bv_t, wo_t, out_sh):
    from contextlib import ExitStack
    ctx = ExitStack()
    with ctx:
        const = ctx.enter_context(tc.tile_pool(name="const", bufs=1))
        wpool = ctx.enter_context(tc.tile_pool(name="wpool", bufs=1))
        big = ctx.enter_context(tc.tile_pool(name="big", bufs=1))
        dram = ctx.enter_context(tc.tile_pool(name="dram", bufs=1, space="DRAM"))

        # ---------- constants ----------
        identity = const.tile([128, 128], BF)
        make_identity(nc, identity)

        # 0/1 lower-triangle mask for the one partial 128x128 diagonal tile
        # of S^T: keep [jj, ii] iff ii - jj >= 0.
        tri = const.tile([128, 128], BF)
        nc.gpsimd.memset(tri, 1.0)
        nc.gpsimd.affine_select(
            out=tri, in_=tri,
            compare_op=mybir.AluOpType.is_ge, fill=0.0,
            base=0, pattern=[[1, 128]], channel_multiplier=-1,
        )

        eps_t = const.tile([128, 1], FP)
        nc.vector.memset(eps_t, EPS)

        bq_b = const.tile([128, 1], FP)
        bk_b = const.tile([128, 1], FP)
        bv_b = const.tile([128, 1], FP)
        nc.sync.dma_start(out=bq_b, in_=bq_t.ap())
        nc.sync.dma_start(out=bk_b, in_=bk_t.ap())
        nc.sync.dma_start(out=bv_b, in_=bv_t.ap())

        # ---------- comm bounce buffers ----------
        ag_in = dram.tile([NSTRIP, KT, 128, 128], BF)
        ag_out = [dram.tile([NCORES, 2, KT, 128, 128], BF,
                            addr_space="Shared", name=f"ag_out{g}")
                  for g in range(2)]
        a2a_in = dram.tile([NSTRIP, NCORES, 128, 128], BF)
        a2a_out = [dram.tile([NCORES, 128, 128], BF,
                             name=f"a2a_out{r}") for r in range(NSTRIP)]
        dummy_in = dram.tile([16], BF)
        dummy_out = dram.tile([NCORES, 16], BF, addr_space="Shared")

        wq_sb = wpool.tile([128, KT, HPC * DH], BF)
        wk_sb = wpool.tile([128, KT, HPC * DH], BF)
        wv_sb = wpool.tile([128, KT, HPC * DH], BF)
        wo_sb = wpool.tile([128, KT, D], BF)

        # tiny dummy collective: pays the first-op ncfw wakeup (~11us) and
        # the inter-core entry barrier while LayerNorm runs
        nc.sync.dma_start(out=dummy_in, in_=identity[0:1, 0:16])
        nc.gpsimd.collective_compute(
            "AllGather", mybir.AluOpType.bypass,
            replica_groups=[list(range(NCORES))],
            ins=[dummy_in.opt()], outs=[dummy_out.opt()])

        # ---------- phase A: per-strip LayerNorm + transpose + AllGather ---
        with tc.tile_pool(name="lnp", bufs=2) as lnp, \
             tc.tile_pool(name="lns", bufs=2) as lns, \
             tc.tile_pool(name="tstage", bufs=2) as tstage:
            nc.sync.dma_start(
                out=wq_sb,
                in_=wq_t.ap().rearrange("(k p) c -> p k c", p=128))
            nc.sync.dma_start(
                out=wk_sb,
                in_=wk_t.ap().rearrange("(k p) c -> p k c", p=128))
            nc.sync.dma_start(
                out=wv_sb,
                in_=wv_t.ap().rearrange("(k p) c -> p k c", p=128))
            x_tiles = []
            for r in range(NSTRIP):
                x_t = lnp.tile([128, D], FP, tag=f"x{r}", name=f"x_{r}",
                               bufs=1)
                nc.sync.dma_start(out=x_t, in_=x_sh.ap()[ts(r, 128), :])
                x_tiles.append(x_t)
            for r in range(NSTRIP):
                x_t = x_tiles[r]
                stats = lns.tile([128, 2, 6], FP, tag="stats")
                xg = x_t.rearrange("p (s f) -> p s f", f=512)
                for s in range(2):
                    nc.vector.bn_stats(out=stats[:, s, :], in_=xg[:, s, :])
                mv = lns.tile([128, 2], FP, tag="mv")
                nc.vector.bn_aggr(out=mv, in_=stats)
                rstd = lns.tile([128, 1], FP, tag="rstd")
                nc.scalar.activation(out=rstd, in_=mv[:, 1:2],
                                     func=mybir.ActivationFunctionType.Sqrt,
                                     bias=eps_t, scale=1.0)
                nc.vector.reciprocal(out=rstd, in_=rstd)
                xn_bf = lnp.tile([128, D], BF, tag="xnbf")
                nc.vector.tensor_scalar(
                    out=xn_bf, in0=x_t, scalar1=mv[:, 0:1], scalar2=rstd,
                    op0=mybir.AluOpType.subtract, op1=mybir.AluOpType.mult)
                xnT_s = tstage.tile([128, KT, 128], BF, tag="xnT")
                nc.scalar.dma_start_transpose(out=xnT_s, in_=xn_bf)
                nc.sync.dma_start(
                    out=ag_in[r].rearrange("k p t -> p k t"),
                    in_=xnT_s)
                if r % 2 == 1:
                    # gather strips (r-1, r) = tokens [1024*(r-1), 1024*(r+1))
                    nc.gpsimd.collective_compute(
                        "AllGather", mybir.AluOpType.bypass,
                        replica_groups=[list(range(NCORES))],
                        ins=[ag_in[r - 1:r + 1].opt()],
                        outs=[ag_out[r // 2].opt()])

        # ---------- phase B+C: QKV + interleaved attention + A2A/out-proj --
        qT = big.tile([128, TCH, 512], BF)   # rows: [h0 64 | h1 64]
        kTt = big.tile([128, TCH, 512], BF)
        vhat = big.tile([128, JTB * B, 130], BF)  # col 64/129 = 1
        nc.gpsimd.memset(vhat[:, :, 64:65], 1.0)
        nc.gpsimd.memset(vhat[:, :, 129:130], 1.0)
        outT = [big.tile([64, T], BF, name=f"outT{h}") for h in range(HPC)]
        a2a_sb = big.tile([128, NCORES, 512], BF)
        out_view = out_sh.ap().rearrange("(t p) e -> p t e", p=128)

        def a2a_stage(q):
            # ship outT column strip {1024*q + 128*c} to core c
            for c in range(NCORES):
                for h in range(HPC):
                    src = outT[h].rearrange(
                        "p (r q t) -> p r q t", r=NSTRIP, q=NCORES)
                    nc.sync.dma_start(
                        out=a2a_in[q, c, ds(64 * h, 64), :],
                        in_=src[:, q, c, :])
            nc.gpsimd.collective_compute(
                "AllToAll", mybir.AluOpType.bypass,
                replica_groups=[list(range(NCORES))],
                ins=[a2a_in[q].opt()], outs=[a2a_out[q].opt()])

        def op_thunks(q):
            # output projection for token tile q (strip q of this core)
            thunks = []

            def load():
                for ct in range(NCORES):
                    nc.sync.dma_start(
                        out=a2a_sb[:, ct, ds(128 * q, 128)],
                        in_=a2a_out[q][ct])
            thunks.append(load)
            for ec in range(D // 512):
                po = qkv_ps.tile([128, 512], FP, tag="acc",
                                 name=f"po_{q}_{ec}")

                def mms(po=po, ec=ec):
                    for ct in range(NCORES):
                        nc.tensor.matmul(
                            po, a2a_sb[:, ct, ds(128 * q, 128)],
                            wo_sb[:, ct, ds(512 * ec, 512)],
                            start=(ct == 0), stop=(ct == NCORES - 1))
                thunks.append(mms)

                def store(po=po, ec=ec):
                    ost = vst_holder[0].tile([128, 512], FP, tag="ost",
                                             name=f"ost_{q}_{ec}")
                    nc.vector.tensor_copy(out=ost, in_=po)
                    nc.sync.dma_start(
                        out=out_view[:, q, ds(512 * ec, 512)], in_=ost)
                thunks.append(store)
            return thunks

        vst_holder = [None]
        with tc.tile_pool(name="xstream", bufs=2) as xstream, \
             tc.tile_pool(name="qkv_ps", bufs=2, space="PSUM") as qkv_ps, \
             tc.tile_pool(name="s_ps", bufs=2, space="PSUM") as s_ps, \
             tc.tile_pool(name="av_ps", bufs=1, space="PSUM") as av_ps, \
             tc.tile_pool(name="espool", bufs=6) as espool, \
             tc.tile_pool(name="smallp", bufs=4) as smallp, \
             tc.tile_pool(name="ostp", bufs=3) as ostp, \
             tc.tile_pool(name="vstage", bufs=2) as vst:
            vst_holder[0] = ostp
            pending_att = None
            for r in range(NSTRIP):
                g, rr = r // 2, r % 2
                xts = []
                for u in range(2):
                    xtu = xstream.tile([128, KT, 512], BF, tag=f"xt{u}",
                                       name=f"xt_{r}_{u}")
                    for cc in range(4):
                        c = 4 * u + cc
                        nc.sync.dma_start(
                            out=xtu[:, :, ds(128 * cc, 128)],
                            in_=ag_out[g][c, rr].rearrange("k p t -> p k t"))
                    xts.append(xtu)
                if r == 1:
                    nc.sync.dma_start(
                        out=wo_sb,
                        in_=wo_t.ap().rearrange("(k p) e -> p k e", p=128))
                for u in range(2):
                    tci = 2 * r + u
                    qkv = _qkv_thunks(nc, tci, xts[u],
                                      wq_sb, wk_sb, wv_sb, bq_b, bk_b, bv_b,
                                      qT, kTt, vhat, qkv_ps, vst)
                    if pending_att is None:
                        seq = qkv
                    else:
                        seq = _merge(pending_att, qkv)
                    for thunk in seq:
                        thunk()
                    # attention for chunks (2q, 2q+1) fully emitted at
                    # tci == 2q+2: ship strip q's outputs
                    if tci >= 2 and tci % 2 == 0:
                        a2a_stage(tci // 2 - 1)
                    # out-proj for quarter q appended once its AllToAll has
                    # had ~2 chunks of compute to land (not merged: a
                    # not-yet-ready op matmul would stall the FIFO PE queue)
                    if tci in (5, 6, 7):
                        for thunk in op_thunks(tci - 5):
                            thunk()
                    b_, ic = tci // ICB, tci % ICB
                    pending_att = _att_thunks(nc, b_, ic, kTt, qT, vhat, tri,
                                              outT, s_ps, av_ps, espool,
                                              smallp)
            for thunk in pending_att:
                thunk()
            a2a_stage(3)
            for thunk in op_thunks(3):
                thunk()


_NC = None
LAST_EXEC_TIME_NS = None


def _get_nc():
    global _NC
    if _NC is None:
        _NC = build()
    return _NC


def make_in_maps(x, gamma, beta, w_qkv, w_out):
    bf = ml_dtypes.bfloat16
    x = np.ascontiguousarray(np.asarray(x, dtype=np.float32)).reshape(T, D)
    gamma = np.asarray(gamma, dtype=np.float32)
    beta = np.asarray(beta, dtype=np.float32)
    w_qkv = np.asarray(w_qkv, dtype=np.float32)
    qkv_bias = beta @ w_qkv                      # [3*INNER]
    w_qkv = (w_qkv * gamma[:, None]).astype(bf)  # fold gamma into weights
    w_out = np.ascontiguousarray(np.asarray(w_out, dtype=np.float32).astype(bf))
    xs = x.reshape(NSTRIP, NCORES, 128, D)
    in_maps = []
    for c in range(NCORES):
        cols = slice(128 * c, 128 * c + 128)
        in_maps.append({
            "x_shard": np.ascontiguousarray(xs[:, c].reshape(TS, D)),
            "wq": np.ascontiguousarray(w_qkv[:, cols]),
            "wk": np.ascontiguousarray(w_qkv[:, INNER:][:, cols]),
            "wv": np.ascontiguousarray(w_qkv[:, 2 * INNER:][:, cols]),
            "bq": np.ascontiguousarray(qkv_bias[cols].reshape(128, 1)),
            "bk": np.ascontiguousarray(qkv_bias[INNER:][cols].reshape(128, 1)),
            "bv": np.ascontiguousarray(
                qkv_bias[2 * INNER:][cols].reshape(128, 1)),
            "w_out": w_out,
        })
    return in_maps


def kernel(x, mask, gamma, beta, w_qkv, w_out):
    global LAST_EXEC_TIME_NS
    nc = _get_nc()
    in_maps = make_in_maps(x, gamma, beta, w_qkv, w_out)
    res = run_bass_kernel_spmd(nc, in_maps, core_ids=list(range(NCORES)))
    LAST_EXEC_TIME_NS = res.exec_time_ns
    out = np.zeros((NSTRIP, NCORES, 128, D), dtype=np.float32)
    for c in range(NCORES):
        out[:, c] = res.results[c]["out_shard"].reshape(NSTRIP, 128, D)
    return out.reshape(B, N, D).astype(np.float32)


# revision 30
# speedup vs baseline: 1.1935x; 1.0784x over previous
"""Distributed Trainium2 kernel: LayerNorm -> QKV -> causal MHA -> out-proj.

Sharding (8 cores):
  - LayerNorm + final projection: token-parallel, strip-interleaved: core c
    owns token strips {1024*r + 128*c + [0,128) : r=0..3} (4096 tokens ->
    512/core).  Strip interleaving lets the post-LN AllGather be split into
    4 pipelined 2MiB collectives, each gathering one contiguous 1024-token
    block, so QKV/attention compute starts after the first lands.
  - Attention + QKV: head-parallel (16 heads -> 2/core).
  - Comms: 4x AllGather of xn^T strips (bf16) after LN; 2x AllToAll of
    per-head attention output (one per 2048-token half) re-shards the
    inner-dim contraction to token-parallel; no AllReduce needed.

Layout notes:
  - All activations are kept TRANSPOSED ([feature, token]) so every matmul
    contraction runs over the partition axis.  S is computed transposed
    (S^T[j,i] = k_j . q_i); softmax sums come from an appended ones-column
    on V (m=65 matmul); the dh^-0.5 scale rides the exp activation's free
    affine.  Streams are causally trimmed at 128 granularity; the single
    partial diagonal 128x128 tile is zeroed with a triangle mask post-exp.
  - The two heads' K=64 S-matmuls are emitted back-to-back at row-disjoint
    tile_position (0,0)/(64,0) so they co-execute on the PE array.
  - gamma/beta are folded host-side into w_qkv (row scale) and per-feature
    biases (beta @ w_qkv), so LN on-device is just (x-mu)*rstd.
  - Matmul inputs are bf16 (4x the fp32 TensorE rate); accumulation fp32.
  - Attention groups are interleaved into the QKV token-chunk loop so
    ScalarE exp work overlaps TensorE QKV matmuls and the PE stream stays
    dense (HAM-warm).  The first AllToAll + its out-projection are
    interleaved into the attention tail.
"""

import numpy as np
import ml_dtypes

import concourse.bass as bass
import concourse.tile as tile
from concourse import bacc, mybir
from concourse.bass import ds, ts
from concourse.bass_utils import run_bass_kernel_spmd
from concourse.masks import make_identity

B, N, D = 2, 2048, 1024
HEADS, DH = 16, 64
INNER = HEADS * DH          # 1024
NCORES = 8
T = B * N                   # 4096 tokens
TS = T // NCORES            # 512 tokens per core
HPC = HEADS // NCORES       # 2 heads per core
SCALE = float(DH) ** -0.5   # 0.125
EPS = 1e-5

FP = mybir.dt.float32
BF = mybir.dt.bfloat16

KT = D // 128               # 8 contraction tiles of 128 over dim
TCH = T // 512              # 8 token chunks of 512
ICB = N // 512              # 4 i-chunks of 512 per batch
JTB = N // 128              # 16 j-tiles of 128 per batch
NSTRIP = 4                  # AllGather rounds; round r = tokens [1024r,1024r+1024)


def build():
    nc = bacc.Bacc("TRN2", target_bir_lowering=False, debug=False,
                   num_devices=NCORES)

    x_sh = nc.dram_tensor("x_shard", [TS, D], FP, kind="ExternalInput")
    wq_t = nc.dram_tensor("wq", [D, HPC * DH], BF, kind="ExternalInput")
    wk_t = nc.dram_tensor("wk", [D, HPC * DH], BF, kind="ExternalInput")
    wv_t = nc.dram_tensor("wv", [D, HPC * DH], BF, kind="ExternalInput")
    bq_t = nc.dram_tensor("bq", [HPC * DH, 1], FP, kind="ExternalInput")
    bk_t = nc.dram_tensor("bk", [HPC * DH, 1], FP, kind="ExternalInput")
    bv_t = nc.dram_tensor("bv", [HPC * DH, 1], FP, kind="ExternalInput")
    wo_t = nc.dram_tensor("w_out", [INNER, D], BF, kind="ExternalInput")
    out_sh = nc.dram_tensor("out_shard", [TS, D], FP, kind="ExternalOutput")

    with tile.TileContext(nc) as tc:
        _body(nc, tc, x_sh, wq_t, wk_t, wv_t, bq_t, bk_t, bv_t, wo_t, out_sh)

    nc.compile()
    return nc


def _att_thunks(nc, b, ic, kTt, qT, vhat, tri, outT,
                s_ps, av_ps, espool, smallp):
    """Attention for query chunk (b, ic) as a list of PE-ordered thunks.

    Each jp step emits the two heads' S matmuls in a single thunk pair so
    the row-disjoint (tile_position 0 / 64) K=64 matmuls stay adjacent in
    the PE queue and co-execute.  S-matmuls for step jp are emitted before
    the AV-matmuls of step jp-1 so the PE never waits on ACT exp latency.
    """
    q_idx = b * ICB + ic
    njt = 4 * (ic + 1)
    av = [av_ps.tile([128, 512], FP, tag=f"av{h}", name=f"av{h}_{q_idx}")
          for h in range(HPC)]
    es = {}

    def i0_of(jt):
        m = jt - 4 * ic
        return 128 * m if m > 0 else 0

    def s_pair(jp, u):
        # Both heads' S^T for j-tile jt land in ONE PSUM tile (h0 cols
        # 0-511, h1 cols 512-1023) so a single exp releases the slot and
        # the scheduler cannot split the co-executing pair.
        def run():
            sx = s_ps.tile([128, 1024], FP, tag="sx",
                           name=f"sx_{q_idx}_{jp}_{u}")
            es[(jp, u)] = (sx, None)
            jt = 2 * jp + u
            tq = b * ICB + jt // 4
            jo = 128 * (jt % 4)
            i0 = i0_of(jt)
            for h in range(HPC):
                nc.tensor.matmul(
                    sx[:, ds(512 * h + i0, 512 - i0)],
                    kTt[ds(64 * h, 64), tq, ds(jo, 128)],
                    qT[ds(64 * h, 64), q_idx, ds(i0, 512 - i0)],
                    start=True, stop=True,
                    tile_position=(64 * h, 0))
        return run

    def exp_t(jp, u):
        def run():
            sx, _ = es[(jp, u)]
            e = espool.tile([128, 1024], BF, tag="es",
                            name=f"es_{q_idx}_{jp}_{u}")
            es[(jp, u)] = (sx, e)
            i0 = i0_of(2 * jp + u)
            if i0 == 0:
                nc.scalar.activation(
                    out=e, in_=sx,
                    func=mybir.ActivationFunctionType.Exp, scale=SCALE)
            else:
                nc.scalar.activation(
                    out=e.rearrange("p (h i) -> p h i", h=2)[:, :, i0:],
                    in_=sx.rearrange("p (h i) -> p h i", h=2)[:, :, i0:],
                    func=mybir.ActivationFunctionType.Exp, scale=SCALE)
        return run

    def av_mm(h, jp, u):
        def run():
            _, e = es[(jp, u)]
            jt = 2 * jp + u
            m = jt - 4 * ic
            i0 = i0_of(jt)
            if 0 <= m < 4:
                # zero the strict upper triangle of the diagonal 128x128 tile
                nc.vector.tensor_tensor(
                    out=e[:, ds(512 * h + 128 * m, 128)],
                    in0=e[:, ds(512 * h + 128 * m, 128)],
                    in1=tri,
                    op=mybir.AluOpType.mult)
            nc.tensor.matmul(
                av[h][0:65, ds(i0, 512 - i0)],
                vhat[:, b * JTB + jt, ds(65 * h, 65)],
                e[:, ds(512 * h + i0, 512 - i0)],
                start=(jt == 0), stop=(jt == njt - 1))
        return run

    def norm(h):
        def run():
            rsum = smallp.tile([1, 512], FP, tag="rsum", name=f"rs{h}_{q_idx}")
            nc.vector.tensor_copy(out=rsum, in_=av[h][64:65, :])
            rec = smallp.tile([1, 512], FP, tag="rec", name=f"rc{h}_{q_idx}")
            nc.vector.reciprocal_approx_fast(out=rec, in_=rsum)
            bc = smallp.tile([64, 512], FP, tag="bc", name=f"bc{h}_{q_idx}")
            nc.gpsimd.partition_broadcast(bc, rec)
            nc.vector.tensor_tensor(
                out=outT[h][:, ds(512 * q_idx, 512)],
                in0=av[h][0:64, :], in1=bc,
                op=mybir.AluOpType.mult)
        return run

    def s_group(jp):
        return [s_pair(jp, 0), s_pair(jp, 1), exp_t(jp, 0), exp_t(jp, 1)]

    def av_group(jp):
        return [av_mm(0, jp, 0), av_mm(0, jp, 1),
                av_mm(1, jp, 0), av_mm(1, jp, 1)]

    thunks = []
    nps = njt // 2
    thunks.extend(s_group(0))
    for jp in range(1, nps):
        thunks.extend(s_group(jp))
        thunks.extend(av_group(jp - 1))
    thunks.extend(av_group(nps - 1))
    thunks.append(norm(0))
    thunks.append(norm(1))
    return thunks


def _qkv_thunks(nc, tci, xt_u, wq_sb, wk_sb, wv_sb, bq_b, bk_b, bv_b,
                qT, kTt, vhat, qkv_ps, vst):
    """QKV projection for token chunk tci as a list of PE-ordered thunks."""
    thunks = []

    def mk_group(w_sb, nm):
        acc = qkv_ps.tile([128, 512], FP, tag="acc", name=f"acc{nm}_{tci}")

        def mm(k):
            def run():
                nc.tensor.matmul(acc, w_sb[:, k, :], xt_u[:, k, :],
                                 start=(k == 0), stop=(k == KT - 1))
            return run
        return acc, mm

    for w_sb, bias, dst, nm in ((wq_sb, bq_b, qT, "q"),
                                (wk_sb, bk_b, kTt, "k")):
        acc, mm = mk_group(w_sb, nm)
        for k in range(KT):
            thunks.append(mm(k))

        def copy(acc=acc, bias=bias, dst=dst, tci=tci):
            nc.vector.tensor_scalar(
                out=dst[:, tci, :], in0=acc, scalar1=bias, scalar2=None,
                op0=mybir.AluOpType.add)
        thunks.append(copy)

    accv, mmv = mk_group(wv_sb, "v")
    for k in range(KT):
        thunks.append(mmv(k))
    vs = vst.tile([128, 512], BF, tag="vs", name=f"vs_{tci}")

    def vcopy():
        nc.vector.tensor_scalar(
            out=vs, in0=accv, scalar1=bv_b, scalar2=None,
            op0=mybir.AluOpType.add)
    thunks.append(vcopy)

    def vtrans():
        def run():
            vstg = vst.tile([128, 4, 128], BF, tag="vstg", name=f"vstg_{tci}")
            nc.sync.dma_start_transpose(out=vstg, in_=vs)
            nc.vector.tensor_copy(out=vhat[:, ds(tci * 4, 4), 0:64],
                                  in_=vstg[:, :, 0:64])
            nc.vector.tensor_copy(out=vhat[:, ds(tci * 4, 4), 65:129],
                                  in_=vstg[:, :, 64:128])
        return run
    thunks.append(vtrans())
    return thunks


def _merge(primary, filler):
    """Interleave filler thunks evenly between primary thunks."""
    out = []
    np_, nf = len(primary), len(filler)
    fi = 0
    for i, p in enumerate(primary):
        out.append(p)
        want = (i + 1) * nf // np_
        while fi < want:
            out.append(filler[fi])
            fi += 1
    out.extend(filler[fi:])
    return out


def _body(nc, tc, x_sh, wq_t, wk_t, wv_t, bq_t, bk_t, bv_t, wo_t, out_sh):
    from contextlib import ExitStack
    ctx = ExitStack()
    with ctx:
        const = ctx.enter_context(tc.tile_pool(name="const", bufs=1))
        wpool = ctx.enter_context(tc.tile_pool(name="wpool", bufs=1))
        big = ctx.enter_context(tc.tile_pool(name="big", bufs=1))
        dram = ctx.enter_context(tc.tile_pool(name="dram", bufs=1, space="DRAM"))

        # ---------- constants ----------
        identity = const.tile([128, 128], BF)
        make_identity(nc, identity)

        # 0/1 lower-triangle mask for the one partial 128x128 diagonal tile
        # of S^T: keep [jj, ii] iff ii - jj >= 0.
        tri = const.tile([128, 128], BF)
        nc.gpsimd.memset(tri, 1.0)
        nc.gpsimd.affine_select(
            out=tri, in_=tri,
            compare_op=mybir.AluOpType.is_ge, fill=0.0,
            base=0, pattern=[[1, 128]], channel_multiplier=-1,
        )

        eps_t = const.tile([128, 1], FP)
        nc.vector.memset(eps_t, EPS)

        bq_b = const.tile([128, 1], FP)
        bk_b = const.tile([128, 1], FP)
        bv_b = const.tile([128, 1], FP)
        nc.sync.dma_start(out=bq_b, in_=bq_t.ap())
        nc.sync.dma_start(out=bk_b, in_=bk_t.ap())
        nc.sync.dma_start(out=bv_b, in_=bv_t.ap())

        # ---------- comm bounce buffers ----------
        ag_in = dram.tile([NSTRIP, KT, 128, 128], BF)
        ag_out = dram.tile([NCORES, NSTRIP, KT, 128, 128], BF,
                           addr_space="Shared")
        a2a_in = dram.tile([2, NCORES, 128, 2 * 128], BF)
        a2a_out = [dram.tile([NCORES, 128, 2 * 128], BF,
                             name=f"a2a_out{hh}") for hh in range(2)]

        wq_sb = wpool.tile([128, KT, HPC * DH], BF)
        wk_sb = wpool.tile([128, KT, HPC * DH], BF)
        wv_sb = wpool.tile([128, KT, HPC * DH], BF)
        wo_sb = wpool.tile([128, KT, D], BF)



        # ---------- phase A: per-strip LayerNorm + transpose + AllGather ---
        with tc.tile_pool(name="lnp", bufs=2) as lnp, \
             tc.tile_pool(name="lns", bufs=2) as lns, \
             tc.tile_pool(name="tstage", bufs=2) as tstage:
            nc.sync.dma_start(
                out=wq_sb,
                in_=wq_t.ap().rearrange("(k p) c -> p k c", p=128))
            nc.sync.dma_start(
                out=wk_sb,
                in_=wk_t.ap().rearrange("(k p) c -> p k c", p=128))
            nc.sync.dma_start(
                out=wv_sb,
                in_=wv_t.ap().rearrange("(k p) c -> p k c", p=128))
            x_tiles = []
            for r in range(NSTRIP):
                x_t = lnp.tile([128, D], FP, tag=f"x{r}", name=f"x_{r}",
                               bufs=1)
                nc.sync.dma_start(out=x_t, in_=x_sh.ap()[ts(r, 128), :])
                x_tiles.append(x_t)
            for r in range(NSTRIP):
                x_t = x_tiles[r]
                stats = lns.tile([128, 2, 6], FP, tag="stats")
                xg = x_t.rearrange("p (s f) -> p s f", f=512)
                for s in range(2):
                    nc.vector.bn_stats(out=stats[:, s, :], in_=xg[:, s, :])
                mv = lns.tile([128, 2], FP, tag="mv")
                nc.vector.bn_aggr(out=mv, in_=stats)
                rstd = lns.tile([128, 1], FP, tag="rstd")
                nc.scalar.activation(out=rstd, in_=mv[:, 1:2],
                                     func=mybir.ActivationFunctionType.Sqrt,
                                     bias=eps_t, scale=1.0)
                nc.vector.reciprocal(out=rstd, in_=rstd)
                xn_bf = lnp.tile([128, D], BF, tag="xnbf")
                nc.vector.tensor_scalar(
                    out=xn_bf, in0=x_t, scalar1=mv[:, 0:1], scalar2=rstd,
                    op0=mybir.AluOpType.subtract, op1=mybir.AluOpType.mult)
                xnT_s = tstage.tile([128, KT, 128], BF, tag="xnT")
                nc.scalar.dma_start_transpose(out=xnT_s, in_=xn_bf)
                nc.sync.dma_start(
                    out=ag_in[r].rearrange("k p t -> p k t"),
                    in_=xnT_s)

        nc.gpsimd.collective_compute(
            "AllGather", mybir.AluOpType.bypass,
            replica_groups=[list(range(NCORES))],
            ins=[ag_in.opt()], outs=[ag_out.opt()])

        # ---------- phase B+C: QKV + interleaved attention + A2A/out-proj --
        qT = big.tile([128, TCH, 512], BF)   # rows: [h0 64 | h1 64]
        kTt = big.tile([128, TCH, 512], BF)
        vhat = big.tile([128, JTB * B, 130], BF)  # col 64/129 = 1
        nc.gpsimd.memset(vhat[:, :, 64:65], 1.0)
        nc.gpsimd.memset(vhat[:, :, 129:130], 1.0)
        outT = [big.tile([64, T], BF, name=f"outT{h}") for h in range(HPC)]
        a2a_sb = big.tile([128, NCORES, 512], BF)
        out_view = out_sh.ap().rearrange("(t p) e -> p t e", p=128)

        def a2a_stage(hh):
            # ship outT column strips {1024*(2hh+r') + 128*c} to core c
            for c in range(NCORES):
                for h in range(HPC):
                    src = outT[h].rearrange(
                        "p (r q t) -> p r q t", r=NSTRIP, q=NCORES)
                    nc.sync.dma_start(
                        out=a2a_in[hh, c, ds(64 * h, 64), :].rearrange(
                            "p (r t) -> p r t", r=2),
                        in_=src[:, ds(2 * hh, 2), c, :])
            nc.gpsimd.collective_compute(
                "AllToAll", mybir.AluOpType.bypass,
                replica_groups=[list(range(NCORES))],
                ins=[a2a_in[hh].opt()], outs=[a2a_out[hh].opt()])

        def op_thunks(hh):
            # output projection for token tiles of half hh (strips 2hh,2hh+1)
            thunks = []

            def load():
                for ct in range(NCORES):
                    nc.sync.dma_start(
                        out=a2a_sb[:, ct, ds(256 * hh, 256)],
                        in_=a2a_out[hh][ct])
            thunks.append(load)
            for rr in range(2):
                tt = 2 * hh + rr
                for ec in range(D // 512):
                    po = qkv_ps.tile([128, 512], FP, tag="acc",
                                     name=f"po_{tt}_{ec}")

                    def mms(po=po, tt=tt, ec=ec):
                        for ct in range(NCORES):
                            nc.tensor.matmul(
                                po, a2a_sb[:, ct, ds(128 * tt, 128)],
                                wo_sb[:, ct, ds(512 * ec, 512)],
                                start=(ct == 0), stop=(ct == NCORES - 1))
                    thunks.append(mms)

                    def store(po=po, tt=tt, ec=ec):
                        ost = vst_holder[0].tile([128, 512], FP, tag="ost",
                                                 name=f"ost_{tt}_{ec}")
                        nc.vector.tensor_copy(out=ost, in_=po)
                        nc.sync.dma_start(
                            out=out_view[:, tt, ds(512 * ec, 512)], in_=ost)
                    thunks.append(store)
            return thunks

        vst_holder = [None]
        with tc.tile_pool(name="xstream", bufs=2) as xstream, \
             tc.tile_pool(name="qkv_ps", bufs=2, space="PSUM") as qkv_ps, \
             tc.tile_pool(name="s_ps", bufs=2, space="PSUM") as s_ps, \
             tc.tile_pool(name="av_ps", bufs=1, space="PSUM") as av_ps, \
             tc.tile_pool(name="espool", bufs=6) as espool, \
             tc.tile_pool(name="smallp", bufs=4) as smallp, \
             tc.tile_pool(name="ostp", bufs=3) as ostp, \
             tc.tile_pool(name="vstage", bufs=2) as vst:
            vst_holder[0] = ostp
            pending_att = None
            for r in range(NSTRIP):
                xts = []
                for u in range(2):
                    xtu = xstream.tile([128, KT, 512], BF, tag=f"xt{u}",
                                       name=f"xt_{r}_{u}")
                    for cc in range(4):
                        c = 4 * u + cc
                        nc.sync.dma_start(
                            out=xtu[:, :, ds(128 * cc, 128)],
                            in_=ag_out[c, r].rearrange("k p t -> p k t"))
                    xts.append(xtu)
                if r == 1:
                    nc.sync.dma_start(
                        out=wo_sb,
                        in_=wo_t.ap().rearrange("(k p) e -> p k e", p=128))
                for u in range(2):
                    tci = 2 * r + u
                    qkv = _qkv_thunks(nc, tci, xts[u],
                                      wq_sb, wk_sb, wv_sb, bq_b, bk_b, bv_b,
                                      qT, kTt, vhat, qkv_ps, vst)
                    if pending_att is None:
                        seq = qkv
                    else:
                        seq = _merge(pending_att, qkv)
                    for thunk in seq:
                        thunk()
                    if tci == 4:
                        # batch-0 attention (chunks 0-3) fully emitted
                        a2a_stage(0)
                    if tci == 6:
                        # first-half out-proj: its AllToAll landed long ago
                        for thunk in op_thunks(0):
                            thunk()
                    b_, ic = tci // ICB, tci % ICB
                    pending_att = _att_thunks(nc, b_, ic, kTt, qT, vhat, tri,
                                              outT, s_ps, av_ps, espool,
                                              smallp)
            for thunk in pending_att:
                thunk()
            a2a_stage(1)
            for thunk in op_thunks(1):
                thunk()


_NC = None
LAST_EXEC_TIME_NS = None


def _get_nc():
    global _NC
    if _NC is None:
        _NC = build()
    return _NC


def make_in_maps(x, gamma, beta, w_qkv, w_out):
    bf = ml_dtypes.bfloat16
    x = np.ascontiguousarray(np.asarray(x, dtype=np.float32)).reshape(T, D)
    gamma = np.asarray(gamma, dtype=np.float32)
    beta = np.asarray(beta, dtype=np.float32)
    w_qkv = np.asarray(w_qkv, dtype=np.float32)
    qkv_bias = beta @ w_qkv                      # [3*INNER]
    w_qkv = (w_qkv * gamma[:, None]).astype(bf)  # fold gamma into weights
    w_out = np.ascontiguousarray(np.asarray(w_out, dtype=np.float32).astype(bf))
    xs = x.reshape(NSTRIP, NCORES, 128, D)
    in_maps = []
    for c in range(NCORES):
        cols = slice(128 * c, 128 * c + 128)
        in_maps.append({
            "x_shard": np.ascontiguousarray(xs[:, c].reshape(TS, D)),
            "wq": np.ascontiguousarray(w_qkv[:, cols]),
            "wk": np.ascontiguousarray(w_qkv[:, INNER:][:, cols]),
            "wv": np.ascontiguousarray(w_qkv[:, 2 * INNER:][:, cols]),
            "bq": np.ascontiguousarray(qkv_bias[cols].reshape(128, 1)),
            "bk": np.ascontiguousarray(qkv_bias[INNER:][cols].reshape(128, 1)),
            "bv": np.ascontiguousarray(
                qkv_bias[2 * INNER:][cols].reshape(128, 1)),
            "w_out": w_out,
        })
    return in_maps


def kernel(x, mask, gamma, beta, w_qkv, w_out):
    global LAST_EXEC_TIME_NS
    nc = _get_nc()
    in_maps = make_in_maps(x, gamma, beta, w_qkv, w_out)
    res = run_bass_kernel_spmd(nc, in_maps, core_ids=list(range(NCORES)))
    LAST_EXEC_TIME_NS = res.exec_time_ns
    out = np.zeros((NSTRIP, NCORES, 128, D), dtype=np.float32)
    for c in range(NCORES):
        out[:, c] = res.results[c]["out_shard"].reshape(NSTRIP, 128, D)
    return out.reshape(B, N, D).astype(np.float32)
